# revision 1
# baseline (speedup 1.0000x reference)
import math
from contextlib import ExitStack

import numpy as np

import concourse.bass as bass
import concourse.tile as tile
from concourse import bacc, mybir
from concourse.masks import make_identity

F32 = mybir.dt.float32
BF16 = mybir.dt.bfloat16
AL = mybir.AluOpType
AF = mybir.ActivationFunctionType

C = 192          # channels
HEADS = 4
CH = C // HEADS  # 48
W = 128          # image width
SR = 16          # rows per stripe
PW = W + 2       # padded width
PR = SR + 2      # padded rows per stripe


def host_prep(kv_w, kv_dw_w, q_w, q_dw_w, proj_w, temperature):
    """Host-side weight transforms (all tiny). Returns dict of extra device inputs."""
    kv_w = kv_w.astype(np.float64)
    q_w = q_w.astype(np.float64)
    q_dw_w = q_dw_w.astype(np.float64)
    proj_w = proj_w.astype(np.float64)
    # kv 1x1: lhsT = W^T [c_in, c_out=2C]
    wkvT = kv_w[:, :, 0, 0].T.copy()  # [192, 384]
    # fused dense conv: W_eff[o, j, dy, dx] = sum_i q_dw_w[o,i,dy,dx] * q_w[i,j]
    weff = np.einsum("oiyx,ij->ojyx", q_dw_w, q_w[:, :, 0, 0])  # [192,192,3,3]
    # device layout: weffT[j, tap*192 + o]
    weffT = np.transpose(weff, (1, 2, 3, 0)).reshape(C, 9 * C).copy()
    projT = proj_w[:, :, 0, 0].T.copy()  # [c, o]
    # K-packed pair weights for y ch 128..191: tap idx = (dy+1)*3 + (dx+1)
    lo = weffT[128:192, :].reshape(64, 9, C)
    weffP = np.zeros((128, 5 * C), np.float64)
    for p, dy in enumerate((-1, 0, 1)):       # pairs {(dy,0) lower, (dy,-1) upper}
        weffP[0:64, p * C:(p + 1) * C] = lo[:, (dy + 1) * 3 + 1]
        weffP[64:128, p * C:(p + 1) * C] = lo[:, (dy + 1) * 3 + 0]
    weffP[0:64, 3 * C:4 * C] = lo[:, 5]       # (0,+1) lower
    weffP[64:128, 3 * C:4 * C] = lo[:, 2]     # (-1,+1) upper
    weffP[0:64, 4 * C:5 * C] = lo[:, 8]       # (1,+1) single
    dws = kv_dw_w[:, 0].reshape(2 * C, 9).copy()  # [384, 9]
    dwdiag = np.zeros((128, 2 * 9 * 128), np.float64)
    for mc in range(2):          # kv1 chunks 1 and 2 (channels 128..383)
        for t in range(9):
            col = (9 * mc + t) * 128
            dwdiag[np.arange(128), col + np.arange(128)] = dws[128 * (mc + 1):128 * (mc + 2), t]
    tau = np.repeat(np.asarray(temperature, np.float64).reshape(HEADS), CH) * math.log(CH)
    m = np.full((96, 96), -1e9, np.float32)
    m[0:48, 0:48] = 0.0
    m[48:96, 48:96] = 0.0
    return {
        "bmask": m,
        "wkvT": wkvT.astype(np.float32),
        "weffT": weffT.astype(np.float32),
        "weffP": weffP.astype(np.float32),
        "projT": projT.astype(np.float32),
        "dws": dws.astype(np.float32),
        "dwdiag": dwdiag.astype(np.float32),
        "tau": tau.reshape(C, 1).astype(np.float32),
    }


def build(H=128, debug=False, dw_on_pe=0):
    """Build + compile the per-core program. H = image height (rows)."""
    HW = H * W
    NS = H // SR  # stripes
    NCK = HW // 512  # output chunks

    nc = bacc.Bacc("TRN2", target_bir_lowering=False, debug=debug,
                   enable_asserts=False, num_devices=1)
    x = nc.dram_tensor("x", [C, HW], F32, kind="ExternalInput").ap()
    y = nc.dram_tensor("y", [C, HW], F32, kind="ExternalInput").ap()
    wkvT = nc.dram_tensor("wkvT", [C, 2 * C], F32, kind="ExternalInput").ap()
    weffT = nc.dram_tensor("weffT", [C, 9 * C], F32, kind="ExternalInput").ap()
    weffP = nc.dram_tensor("weffP", [128, 5 * C], F32, kind="ExternalInput").ap()
    projT = nc.dram_tensor("projT", [C, C], F32, kind="ExternalInput").ap()
    dws = nc.dram_tensor("dws", [2 * C, 9], F32, kind="ExternalInput").ap()
    dwdiag = nc.dram_tensor("dwdiag", [128, 2 * 9 * 128], F32, kind="ExternalInput").ap()
    tau = nc.dram_tensor("tau", [C, 1], F32, kind="ExternalInput").ap()
    out = nc.dram_tensor("out", [C, HW], F32, kind="ExternalOutput").ap()
    vscr = nc.dram_tensor("vscr", [C, HW], BF16, kind="Internal").ap()
    rscr = nc.dram_tensor("rscr", [1, C], F32, kind="Internal").ap()
    bmask = nc.dram_tensor("bmask", [96, 96], F32, kind="ExternalInput").ap()

    with tile.TileContext(nc) as tc:
        with ExitStack() as ctx:
            wp = ctx.enter_context(tc.tile_pool(name="wp", bufs=1))       # persistent sbuf
            pp = ctx.enter_context(tc.tile_pool(name="pp", bufs=1, space="PSUM"))  # persistent psum

            # ---- weights to sbuf + bf16 casts (setup) ----
            stg = ctx.enter_context(tc.tile_pool(name="stg", bufs=2))

            def load_cast(ap_dram, p, f, nm):
                t32 = stg.tile([p, f], F32, tag="ldstage", name=f"stage_{nm}")
                nc.sync.dma_start(t32[:], ap_dram)
                tb = wp.tile([p, f], BF16, tag=nm, name=nm)
                nc.gpsimd.tensor_copy(tb[:], t32[:])
                return tb

            wkv_hi = load_cast(wkvT[0:128, :], 128, 2 * C, "wkv_hi")
            wkv_lo = load_cast(wkvT[128:192, :], 64, 2 * C, "wkv_lo")
            weff_hi = load_cast(weffT[0:128, :], 128, 9 * C, "weff_hi")
            weff_pr = load_cast(weffP[:, :], 128, 5 * C, "weff_pr")
            ddg = load_cast(dwdiag[:, :], 128, 2 * 9 * 128, "ddg")
            prA = load_cast(projT[0:96, :], 96, C, "prA")
            prB = load_cast(projT[96:192, :], 96, C, "prB")
            dws_t = wp.tile([128, 9 * 3], F32)  # 3 chunks side by side: [:,9m+t]
            for m in range(3):
                nc.sync.dma_start(dws_t[0:128, 9 * m:9 * m + 9], dws[128 * m:128 * m + 128, :])
            tauA = wp.tile([128, 1], F32)
            nc.sync.dma_start(tauA[:], tau[0:128, :])
            tauB = wp.tile([64, 1], F32)
            nc.sync.dma_start(tauB[:], tau[128:192, :])
            identF = wp.tile([128, 128], F32)
            make_identity(nc, identF[:])

            # ssq accumulator slots (per stripe), fp32
            ssqA = wp.tile([128, NS], F32)   # k ch 0..127
            ssqB = wp.tile([64, NS], F32)    # k ch 128..191

            # persistent psum: attn raw blocks + q gram
            raw01 = pp.tile([96, 96], F32)
            raw23 = pp.tile([96, 96], F32)
            gq_hi = pp.tile([128, 128], F32)
            gq_lo = pp.tile([64, 64], F32)

            taps = [(dy, dx) for dy in (-1, 0, 1) for dx in (-1, 0, 1)]

            with ExitStack() as sctx:
                sp = sctx.enter_context(tc.tile_pool(name="sp", bufs=2))      # stripe transients
                qp = sctx.enter_context(tc.tile_pool(name="qp", bufs=4))
                kvpool = sctx.enter_context(tc.tile_pool(name="kvp", bufs=3))
                pk = sctx.enter_context(tc.tile_pool(name="pk", bufs=2, space="PSUM"))
                pq = sctx.enter_context(tc.tile_pool(name="pq", bufs=2, space="PSUM"))

                pend = None
                for s in range(NS):
                    r_lo = SR * s - 1           # first (halo) image row
                    n_lo = r_lo * W

                    # ---- x/y DMA + cast (+pad for y) ----
                    def stage_in(src, name, dmaeng=nc.sync, b_chunk=True):
                        tA = sp.tile([128, PR * W], F32, tag="stA32", name=f"{name}A32")
                        tB = sp.tile([64, PR * W], F32, tag="stB32", name=f"{name}B32")
                        chunks = ((tA, 0, 128), (tB, 128, 64)) if b_chunk else ((tA, 0, 128),)
                        for t, p0, p in chunks:
                            if s == 0:
                                nc.gpsimd.memset(t[:, 0:W], 0.0)
                                dmaeng.dma_start(t[:, W:], src[p0:p0 + p, 0:(PR - 1) * W])
                            elif s == NS - 1:
                                nc.gpsimd.memset(t[:, (PR - 1) * W:], 0.0)
                                dmaeng.dma_start(t[:, 0:(PR - 1) * W], src[p0:p0 + p, n_lo:n_lo + (PR - 1) * W])
                            else:
                                dmaeng.dma_start(t[:], src[p0:p0 + p, n_lo:n_lo + PR * W])
                        return (tA, tB) if b_chunk else tA

                    def stage_in_b19(src, dmaeng):
                        # rows 16s-2 .. 16s+16 (19 rows) for the row-shifted dup
                        t = sp.tile([64, (PR + 1) * W], F32, tag="stB32", name="yB19")
                        lo2 = (SR * s - 2) * W
                        a, b = 0, (PR + 1) * W
                        if s == 0:
                            nc.gpsimd.memset(t[:, 0:2 * W], 0.0)
                            a = 2 * W
                        if s == NS - 1:
                            nc.gpsimd.memset(t[:, (PR + 1 - 1) * W:], 0.0)
                            b = (PR + 1 - 1) * W
                        dmaeng.dma_start(t[:, a:b], src[128:192, lo2 + a:lo2 + b])
                        return t

                    xA32, xB32 = stage_in(x, "x")
                    xbA = sp.tile([128, PR * W], BF16, tag="xbA")
                    nc.gpsimd.tensor_copy(xbA[:], xA32[:])
                    xbB = sp.tile([64, PR * W], BF16, tag="xbB")
                    nc.gpsimd.tensor_copy(xbB[:], xB32[:])

                    yA32 = stage_in(y, "y", nc.scalar, b_chunk=False)
                    yB19 = stage_in_b19(y, nc.scalar)
                    ypA = sp.tile([128, PR, PW], BF16, tag="ypA")
                    nc.gpsimd.memset(ypA[:, :, 0:1], 0.0)
                    nc.gpsimd.memset(ypA[:, :, PW - 1:PW], 0.0)
                    nc.gpsimd.tensor_copy(
                        ypA[:, :, 1:1 + W],
                        yA32[:].rearrange("p (a b) -> p a b", b=W))
                    # ypB1: lower = unshifted, upper = shifted +1 col
                    # ypB2: lower = unshifted, upper = shifted +1 row
                    ypB1 = sp.tile([128, PR, PW], BF16, tag="ypB1")
                    ypB2 = sp.tile([128, PR, PW], BF16, tag="ypB2")
                    yb = yB19[:].rearrange("p (a b) -> p a b", b=W)
                    nc.gpsimd.memset(ypB1[:, :, 0:2], 0.0)
                    nc.gpsimd.memset(ypB1[0:64, :, PW - 1:PW], 0.0)
                    nc.gpsimd.memset(ypB2[:, :, PW - 1:PW], 0.0)
                    nc.gpsimd.tensor_copy(ypB1[0:64, :, 1:1 + W], yb[:, 1:1 + PR, :])
                    nc.gpsimd.tensor_copy(ypB1[64:128, :, 2:2 + W], yb[:, 1:1 + PR, :])
                    nc.gpsimd.tensor_copy(ypB2[0:64, :, 1:1 + W], yb[:, 1:1 + PR, :])
                    nc.gpsimd.tensor_copy(ypB2[64:128, :, 1:1 + W], yb[:, 0:PR, :])

                    # ---- kv 1x1 conv -> padded kvp chunks ----
                    kvp = []
                    for m in range(3):
                        kvt = kvpool.tile([128, PR, PW], BF16, tag=f"kvp{m}")
                        nc.gpsimd.memset(kvt[:, :, 0:1], 0.0)
                        nc.gpsimd.memset(kvt[:, :, PW - 1:PW], 0.0)
                        kvp.append(kvt)
                        lhs_hi = wkv_hi[:, 128 * m:128 * m + 128]
                        lhs_lo = wkv_lo[:, 128 * m:128 * m + 128]
                        for j in range(0, PR * W, 512):
                            w_ = min(512, PR * W - j)
                            pst = pk.tile([128, 512], F32, tag="pkv", name="pkv")
                            ps = pst[:, 0:w_]
                            nc.tensor.matmul(ps, lhs_hi, xbA[:, j:j + w_], start=True, stop=False)
                            nc.tensor.matmul(ps, lhs_lo, xbB[:, j:j + w_], start=False, stop=True)
                            nc.scalar.copy(
                                kvt[:, j // W:j // W + w_ // W, 1:1 + W],
                                ps.rearrange("p (a b) -> p a b", b=W))

                    # ---- depthwise 3x3 (DVE rows 0..9, GPSIMD rows 10..15) ----
                    kA = sp.tile([128, SR, W], BF16, tag="kA")
                    kvmid = sp.tile([128, SR, W], BF16, tag="kvmid")
                    vB = sp.tile([128, SR, W], BF16, tag="vB")
                    douts = [kA, kvmid, vB]
                    for m in range(3 - dw_on_pe):
                        dst = douts[m]
                        for ti, (dy, dx) in enumerate(taps):
                            sc = dws_t[:, 9 * m + ti:9 * m + ti + 1]
                            src = kvp[m][:, 1 + dy:1 + SR + dy, 1 + dx:1 + dx + W]
                            d = dst[:, :, :]
                            if ti == 0:
                                nc.vector.tensor_scalar_mul(d, src, sc)
                            else:
                                tmp = sp.tile([128, SR, W], BF16, tag=f"dwtmp", name="dwtmp")
                                nc.vector.tensor_scalar_mul(tmp[:], src, sc)
                                nc.vector.tensor_add(d, d, tmp[:])
                    for m in range(3 - dw_on_pe, 3):
                        dst = douts[m]
                        for jr in range(0, SR, 4):
                            pdw = pk.tile([128, 512], F32, tag="pkv", name="pdw")
                            for ti, (dy, dx) in enumerate(taps):
                                lhs = ddg[:, (9 * (m - 1) + ti) * 128:(9 * (m - 1) + ti + 1) * 128]
                                rhs = kvp[m][:, 1 + jr + dy:1 + jr + dy + 4, 1 + dx:1 + dx + W]
                                nc.tensor.matmul(pdw[:], lhs, rhs, start=(ti == 0), stop=(ti == 8))
                            nc.vector.tensor_copy(
                                dst[:, jr:jr + 4, :],
                                pdw[:].rearrange("p (a b) -> p a b", b=W))

                    # ---- ssq_k ----
                    scr = sp.tile([128, SR, W], BF16, tag="dwtmp", name="scr")
                    nc.scalar.activation(scr[:], kA[:], AF.Square, accum_out=ssqA[:, s:s + 1])
                    nc.scalar.activation(scr[0:64], kvmid[0:64], AF.Square, accum_out=ssqB[:, s:s + 1])

                    # ---- v spill ----
                    nsl = slice(SR * s * W, SR * s * W + SR * W)
                    nc.sync.dma_start(vscr[0:64, nsl], kvmid[64:128].rearrange("p a b -> p (a b)"))
                    nc.sync.dma_start(vscr[64:192, nsl], vB[:].rearrange("p a b -> p (a b)"))

                    # ---- k transpose: kt[p, r, c] = k[c, r*W + p] ----
                    kt = sp.tile([128, SR, C], BF16, tag="kt")
                    nc.sync.dma_start_transpose(kt[:, :, 0:128], kA[:].rearrange("p a b -> p (a b)"))
                    nc.sync.dma_start_transpose(kt[:, :, 128:192], kvmid[0:64].rearrange("p a b -> p (a b)"))

                    # ---- dense conv (fused q) + attn/gram accumulation ----
                    def attn_mms(qsb, r, kt_):
                        f = (s == 0 and r == 0)
                        l = (s == NS - 1 and r == SR - 1)
                        nc.tensor.matmul(gq_hi[:], qsb[:, 0:128], qsb[:, 0:128], start=f, stop=l)
                        nc.tensor.matmul(gq_lo[:], qsb[:, 128:192], qsb[:, 128:192], start=f, stop=l)
                        nc.tensor.matmul(raw01[:], qsb[:, 0:96], kt_[:, r, 0:96], start=f, stop=l)
                        nc.tensor.matmul(raw23[:], qsb[:, 96:192], kt_[:, r, 96:192], start=f, stop=l)

                    for r in range(SR):
                        psq = pq.tile([128, C], F32, tag="psq")
                        first = True
                        for ti, (dy, dx) in enumerate(taps):
                            wsl = slice(C * ti, C * ti + C)
                            lhs_hi = ypA[:, r + 1 + dy, 1 + dx:1 + dx + W]
                            nc.tensor.matmul(psq, lhs_hi, weff_hi[:, wsl], start=first, stop=False)
                            first = False
                        for p, dy in enumerate((-1, 0, 1)):  # pairs {(dy,0), (dy,-1)}
                            nc.tensor.matmul(psq, ypB1[:, r + 1 + dy, 1:1 + W],
                                             weff_pr[:, p * C:(p + 1) * C], start=False, stop=False)
                        nc.tensor.matmul(psq, ypB2[:, r + 1, 2:2 + W],
                                         weff_pr[:, 3 * C:4 * C], start=False, stop=False)
                        nc.tensor.matmul(psq, ypB1[0:64, r + 2, 2:2 + W],
                                         weff_pr[0:64, 4 * C:5 * C], start=False, stop=True)
                        qsb = qp.tile([128, C], BF16, tag="qsb")
                        nc.scalar.copy(qsb[:], psq[:])
                        if pend is not None:
                            attn_mms(*pend)
                            pend = None
                        if r < SR - 1:
                            pend = (qsb, r, kt)
                        else:
                            attn_mms(qsb, r, kt)

            # ================= phase 2: softmax + MT =================
            with ExitStack() as sctx2:
                s2 = sctx2.enter_context(tc.tile_pool(name="s2", bufs=1))
                p2 = sctx2.enter_context(tc.tile_pool(name="p2", bufs=1, space="PSUM"))

                # ssq_q from gram diagonals
                scr2 = s2.tile([128, 128], F32)
                ssqqA = s2.tile([128, 1], F32)
                nc.vector.scalar_tensor_tensor(scr2[:], gq_hi[:], 1.0, identF[:],
                                               AL.mult, AL.mult, accum_out=ssqqA[:])
                scr2b = s2.tile([64, 64], F32)
                ssqqB = s2.tile([64, 1], F32)
                nc.vector.scalar_tensor_tensor(scr2b[:], gq_lo[:], 1.0, identF[0:64, 0:64],
                                               AL.mult, AL.mult, accum_out=ssqqB[:])
                # ssq_k totals
                sskA = s2.tile([128, 1], F32)
                nc.vector.reduce_sum(sskA[:], ssqA[:], axis=mybir.AxisListType.X)
                sskB = s2.tile([64, 1], F32)
                nc.vector.reduce_sum(sskB[:], ssqB[:], axis=mybir.AxisListType.X)

                def rsqrt(dst, src):
                    nc.vector.reciprocal(dst, src)
                    nc.scalar.activation(dst, dst, AF.Sqrt)

                rqA = s2.tile([128, 1], F32, name="rqA")
                rsqrt(rqA[:], ssqqA[:])
                rqB = s2.tile([64, 1], F32, name="rqB")
                rsqrt(rqB[:], ssqqB[:])
                rkA = s2.tile([128, 1], F32, name="rkA")
                rsqrt(rkA[:], sskA[:])
                rkB = s2.tile([64, 1], F32, name="rkB")
                rsqrt(rkB[:], sskB[:])
                # rq * tau
                nc.vector.tensor_mul(rqA[:], rqA[:], tauA[:])
                nc.vector.tensor_mul(rqB[:], rqB[:], tauB[:])

                # rk rows [1, 96] then broadcast [96, 96] via K=1 matmul with ones
                nc.sync.dma_start(rscr[0:1, 0:128].rearrange("a b -> b a"), rkA[:])
                nc.sync.dma_start(rscr[0:1, 128:192].rearrange("a b -> b a"), rkB[:])
                rkrow = s2.tile([1, 192], F32)
                nc.sync.dma_start(rkrow[:], rscr)
                rkrow_b = s2.tile([1, 192], BF16)
                nc.vector.tensor_copy(rkrow_b[:], rkrow[:])
                ones1 = s2.tile([1, 96], BF16)
                nc.vector.memset(ones1[:], 1.0)
                rkb01p = p2.tile([96, 96], F32)
                nc.tensor.matmul(rkb01p[:], ones1[:], rkrow_b[0:1, 0:96], start=True, stop=True)
                rkb23p = p2.tile([96, 96], F32)
                nc.tensor.matmul(rkb23p[:], ones1[:], rkrow_b[0:1, 96:192], start=True, stop=True)

                # logits = raw * (rq*tau) * rk
                l01 = s2.tile([96, 96], F32)
                nc.scalar.activation(l01[:], raw01[:], AF.Copy, scale=rqA[0:96, :])
                nc.vector.tensor_mul(l01[:], l01[:], rkb01p[:])
                l23 = s2.tile([96, 96], F32)
                rq23 = s2.tile([96, 1], F32)
                nc.sync.dma_start(rq23[0:32, :], rqA[96:128, :])
                nc.sync.dma_start(rq23[32:96, :], rqB[:])
                nc.scalar.activation(l23[:], raw23[:], AF.Copy, scale=rq23[:])
                nc.vector.tensor_mul(l23[:], l23[:], rkb23p[:])

                # softmax per head-pair with additive block mask -> blockdiag bd (bf16)
                msk = s2.tile([96, 96], F32)
                nc.sync.dma_start(msk[:], bmask)
                bd01 = s2.tile([96, 96], BF16)
                bd23 = s2.tile([96, 96], BF16)
                for hb, (lt, bd) in enumerate(((l01, bd01), (l23, bd23))):
                    nc.vector.tensor_add(lt[:], lt[:], msk[:])
                    mx = s2.tile([96, 1], F32, tag=f"mx{hb}", name=f"mx{hb}")
                    nc.vector.reduce_max(mx[:], lt[:], axis=mybir.AxisListType.X)
                    nc.vector.tensor_scalar_mul(mx[:], mx[:], -1.0)
                    ex = s2.tile([96, 96], F32, tag=f"ex{hb}", name=f"ex{hb}")
                    rs = s2.tile([96, 1], F32, tag=f"rs{hb}", name=f"rs{hb}")
                    nc.scalar.activation(ex[:], lt[:], AF.Exp, bias=mx[:], accum_out=rs[:])
                    nc.vector.reciprocal(rs[:], rs[:])
                    nc.vector.tensor_scalar_mul(bd[:], ex[:], rs[:])

                # MT[d, o] = sum_c attn[c, d] * projT[c, o]
                mt_hi_p = p2.tile([96, C], F32)
                nc.tensor.matmul(mt_hi_p[:], bd01[:], prA[:], start=True, stop=True)
                mt_lo_p = p2.tile([96, C], F32)
                nc.tensor.matmul(mt_lo_p[:], bd23[:], prB[:], start=True, stop=True)
                mt_hi = wp.tile([96, C], BF16)
                nc.scalar.copy(mt_hi[:], mt_hi_p[:])
                mt_lo = wp.tile([96, C], BF16)
                nc.scalar.copy(mt_lo[:], mt_lo_p[:])

            # ================= phase 3: out = MT.T @ v, streamed =================
            with ExitStack() as sctx3:
                s3 = sctx3.enter_context(tc.tile_pool(name="s3", bufs=3))
                p3 = sctx3.enter_context(tc.tile_pool(name="p3", bufs=2, space="PSUM"))
                for j in range(NCK):
                    nsl = slice(512 * j, 512 * j + 512)
                    vhi = s3.tile([96, 512], BF16, tag="vhi")
                    nc.sync.dma_start(vhi[:], vscr[0:96, nsl])
                    vlo = s3.tile([96, 512], BF16, tag="vlo")
                    nc.sync.dma_start(vlo[:], vscr[96:192, nsl])
                    f1 = p3.tile([128, 512], F32, tag="f1")
                    nc.tensor.matmul(f1[:], mt_hi[:, 0:128], vhi[:], start=True, stop=False)
                    nc.tensor.matmul(f1[:], mt_lo[:, 0:128], vlo[:], start=False, stop=True)
                    f2 = p3.tile([64, 512], F32, tag="f2")
                    nc.tensor.matmul(f2[:], mt_hi[:, 128:192], vhi[:], start=True, stop=False)
                    nc.tensor.matmul(f2[:], mt_lo[:, 128:192], vlo[:], start=False, stop=True)
                    o1 = s3.tile([128, 512], F32, tag="o1")
                    nc.vector.tensor_copy(o1[:], f1[:])
                    o2 = s3.tile([64, 512], F32, tag="o2")
                    nc.scalar.copy(o2[:], f2[:])
                    nc.scalar.dma_start(out[0:128, nsl], o1[:])
                    nc.scalar.dma_start(out[128:192, nsl], o2[:])

    nc.compile()
    return nc


# ======================= harness entry point =======================
B = 8
H = 128
_NC = None


def _get_nc():
    global _NC
    if _NC is None:
        _NC = build(H=H)
    return _NC


def _make_in_maps(inputs):
    x = np.ascontiguousarray(inputs["x"], np.float32)
    y = np.ascontiguousarray(inputs["y"], np.float32)
    prep = host_prep(inputs["kv_w"], inputs["kv_dw_w"], inputs["q_w"],
                     inputs["q_dw_w"], inputs["proj_w"], inputs["temperature"])
    maps = []
    for b in range(B):
        m = {"x": x[b].reshape(C, H * W), "y": y[b].reshape(C, H * W)}
        m.update(prep)
        maps.append(m)
    return maps


def _run(inputs, trace=False, trace_kwargs=None):
    from concourse.bass_utils import run_bass_kernel_spmd
    nc = _get_nc()
    res = run_bass_kernel_spmd(nc, _make_in_maps(inputs), core_ids=list(range(B)),
                               trace=trace, trace_kwargs=trace_kwargs or {})
    out = np.stack([np.asarray(res.results[b]["out"], np.float32).reshape(C, H, W)
                    for b in range(B)])
    return out, res


def kernel(**inputs) -> np.ndarray:
    out, _ = _run(inputs, trace=False)
    return out

